# revision 4
# baseline (speedup 1.0000x reference)
"""Trainium2 Bass kernel for nn_DGLFeatureGAT (dense per-batch GAT over
F=256 feature-nodes, window W=256, H=4 heads, D=8; data-parallel over
batch: 32 batches/core on 8 cores).

Math: alpha = softmax_s(leaky_relu(el_s + er_i)); softmax cancels any
dst-only factor, so Etil[s,i] = max(a_s*h_i, c_s) with a=exp(el),
c=exp(0.2*el), h=exp(0.8*er) — one DVE tensor_scalar per [128,256] tile.

Key performance structure (137 us on HW, vs 167 us baseline):
- x staged per-kc as float32r (no bf16 convert; fp32r matmuls run at
  bf16 speed when the moving side is >=256 wide)
- software pipeline: stage-4 (normalize/project/store) of group g-1 is
  emitted woven into stages 1-3 of group g (recip sits between the two
  8-op et half-blocks so the DVE never head-of-line blocks)
- h-broadcast: one merged DMA ([4,512]->[1,2048], partition-first APs)
  + 8 small int32-bitcast partition_broadcasts (short GpSimd SBUF-port
  locks; DVE PTR ops contend with GpSimd for the shared SBUF port pair)
- y written from per-group [128,1024] slices via the gpsimd DMA queue
- per-head hb slices ready early -> et starts sooner
"""
import sys
import numpy as np

sys.path.insert(0, "/opt/trn_rl_repo")

import concourse.bass as bass  # noqa: E402
import concourse.bacc as bacc  # noqa: E402
import concourse.tile as tile  # noqa: E402
from concourse import mybir  # noqa: E402
from concourse.bass_utils import run_bass_kernel_spmd  # noqa: E402

F32 = mybir.dt.float32
F32R = mybir.dt.float32r
BF16 = mybir.dt.bfloat16
U32 = mybir.dt.uint32

B, W, F, H, D = 256, 256, 256, 4, 8
NCORES = 8
NB = B // NCORES
NEG_SLOPE = 0.2


def build_nc(nb: int = NB):
    nc = bacc.Bacc("TRN2", target_bir_lowering=False, debug=False)

    x = nc.dram_tensor("x", [nb, W, F], F32, kind="ExternalInput")
    c36 = nc.dram_tensor("c36", [W, 36], F32, kind="ExternalInput")
    fcr4 = nc.dram_tensor("fcr4", [W, 4], F32, kind="ExternalInput")
    sel2r = nc.dram_tensor("sel2r", [128, 128], F32, kind="ExternalInput")
    pwt = nc.dram_tensor("pwt", [128, W], F32, kind="ExternalInput")
    y = nc.dram_tensor("y", [nb, W, F], F32, kind="ExternalOutput")

    featpads = [
        nc.alloc_sbuf_tensor(f"featpad{i}", [128, 256], BF16) for i in range(2)
    ]

    with tile.TileContext(nc) as tc:
        with (
            tc.tile_pool(name="consts", bufs=1) as cpool,
            tc.tile_pool(name="xp", bufs=2) as xpool,
            tc.tile_pool(name="yp", bufs=2) as ypool,
            tc.tile_pool(name="hbA", bufs=4) as hbpoolA,
            tc.tile_pool(name="hbB", bufs=4) as hbpoolB,
            tc.tile_pool(name="et", bufs=16) as etpool,
            tc.tile_pool(name="h4", bufs=3) as h4pool,
            tc.tile_pool(name="hr", bufs=6) as hrpool,
            tc.tile_pool(name="ac", bufs=4) as acpool,
            tc.tile_pool(name="nm", bufs=2) as nmpool,
            tc.tile_pool(name="psA", bufs=2, space="PSUM") as psA,
            tc.tile_pool(name="psE", bufs=1, space="PSUM") as psE,
            tc.tile_pool(name="psG", bufs=2, space="PSUM") as psG,
            tc.tile_pool(name="psD", bufs=1, space="PSUM") as psD,
            tc.tile_pool(name="psO", bufs=2, space="PSUM") as psO,
        ):
            # ---- constants
            c36_f32 = cpool.tile([128, 72], F32R)
            nc.sync.dma_start(
                c36_f32[:].rearrange("p (c n) -> p c n", c=2),
                c36.ap().rearrange("(c p) n -> p c n", c=2).bitcast(F32R),
            )
            fcr_f32 = cpool.tile([128, 8], F32R)
            nc.sync.dma_start(
                fcr_f32[:].rearrange("p (c n) -> p c n", c=2),
                fcr4.ap().rearrange("(c p) n -> p c n", c=2).bitcast(F32R),
            )
            sel2r_f32 = cpool.tile([128, 128], F32)
            nc.sync.dma_start(sel2r_f32[:], sel2r.ap())
            pwt_f32 = cpool.tile([128, 256], F32)
            nc.sync.dma_start(pwt_f32[:], pwt.ap())

            sel2r_sb = cpool.tile([128, 128], BF16)
            pwt_sb = cpool.tile([128, 256], BF16)

            for fp in featpads:
                nc.gpsimd.memset(fp.ap(), 0.0)
                ones_view = fp.ap().rearrange(
                    "q (sc h j) -> q sc h j", sc=2, h=4
                )[:, :, :, 8:9]
                nc.gpsimd.memset(ones_view, 1.0)

            x_bf = None
            y_t = None
            pend = []  # (agg, ac_unused, y_t, bm, g)
            ngroups = nb // 2
            for it in range(ngroups + 1):
                g = it
                have_g = g < ngroups
                s4 = pend.pop(0) if it >= 1 else None
                if it == 1:
                    nc.scalar.copy(sel2r_sb[:], sel2r_f32[:])
                    nc.scalar.copy(pwt_sb[:], pwt_f32[:])

                if have_g:
                    bm = [(2 * g + p) % 4 for p in range(2)]
                    if g % 2 == 0:
                        b0 = 2 * g
                        x_bf = []
                        for kc in range(2):
                            x_t = xpool.tile([128, 1024], F32R, tag=f"xt{kc}")
                            for bp in range(2):
                                nc.sync.dma_start(
                                    x_t[:, 512 * bp : 512 * bp + 512].rearrange(
                                        "q (b f) -> q b f", b=2
                                    ),
                                    x.ap()[b0 + 2 * bp : b0 + 2 * bp + 2].rearrange(
                                        "b (kc q) f -> kc q b f", kc=2
                                    )[kc].bitcast(F32R),
                                )
                            x_bf.append(x_t)
                        y_t = ypool.tile([128, 2048], F32, tag="yt")

                    formA = psA.tile([128, 144], F32, tag="formA")
                    er_ps = psE.tile([128, 512], F32, tag="er")
                    agg = psG.tile([128, 512], F32, tag="agg")
                    ac_sb = acpool.tile([128, 32], F32, tag="ac")

                    # --- tensor: formA + er
                    for p in range(2):
                        for fh in range(2):
                            for kc in range(2):
                                xo = 256 * bm[p] + 128 * fh
                                nc.tensor.matmul(
                                    formA[:, 72 * p + 36 * fh :][:, :36],
                                    x_bf[kc][:, xo : xo + 128],
                                    c36_f32[:, 36 * kc : 36 * kc + 36],
                                    start=(kc == 0),
                                    stop=(kc == 1),
                                )
                    for kc in range(2):
                        xo = 256 * bm[0]
                        nc.tensor.matmul(
                            er_ps[0:4, :],
                            fcr_f32[:, 4 * kc : 4 * kc + 4],
                            x_bf[kc][:, xo : xo + 512],
                            start=(kc == 0),
                            stop=(kc == 1),
                        )

                    # --- scalar: exps + h4d + scatter; sync: hr
                    el_view = formA[:].rearrange(
                        "q (p fh n) -> q p fh n", p=2, fh=2
                    )[:, :, :, 32:36]
                    a_view = ac_sb[:, 0:16].rearrange(
                        "q (p fh n) -> q p fh n", p=2, fh=2
                    )
                    c_view = ac_sb[:, 16:32].rearrange(
                        "q (p fh n) -> q p fh n", p=2, fh=2
                    )
                    nc.scalar.activation(
                        a_view, el_view, mybir.ActivationFunctionType.Exp,
                        scale=1.0,
                    )
                    nc.scalar.activation(
                        c_view, el_view, mybir.ActivationFunctionType.Exp,
                        scale=NEG_SLOPE,
                    )
                    h4d = h4pool.tile([4, 512], BF16, tag="h4d")
                    nc.scalar.activation(
                        h4d[:],
                        er_ps[0:4, :],
                        mybir.ActivationFunctionType.Exp,
                        scale=1.0 - NEG_SLOPE,
                    )
                    hr_all = hrpool.tile([1, 2048], BF16, tag="hr")
                    nc.sync.dma_start(
                        hr_all[0:1, :].rearrange("o (h n) -> o h n", h=4),
                        h4d[:].rearrange("h (o n) -> h o n", o=1),
                    )
                    featpad_g = [featpads[p] for p in range(2)]
                    for p in range(2):
                        fa = formA[:, 72 * p : 72 * p + 72].rearrange(
                            "q (fh n) -> q fh n", fh=2
                        )[:, :, 0:32].rearrange("q fh (h d) -> q fh h d", h=4)
                        fp_view = featpad_g[p].ap().rearrange(
                            "q (sc h j) -> q sc h j", sc=2, h=4
                        )[:, :, :, 0:8]
                        nc.scalar.copy(fp_view, fa)

                # --- stage 4 (g-1) early part: num copy + den matmul
                if s4 is not None:
                    agg_s, yt_s, bm_s, gp = s4
                    den = psD.tile([128, 512], F32, tag="den")
                    num_sb = nmpool.tile([128, 512], BF16, tag="num")
                    rinv = nmpool.tile([128, 512], F32, tag="rinv")
                    rinv_bf = nmpool.tile([128, 512], BF16, tag="rinvb")
                    num_sc = nmpool.tile([128, 512], BF16, tag="nsc")
                    nc.scalar.copy(num_sb[:], agg_s[:])
                    nc.tensor.matmul(
                        den[:], sel2r_sb[:], num_sb[:], start=True, stop=True
                    )

                if have_g:
                    hbpool_g = hbpoolA if g % 2 == 0 else hbpoolB
                    hb_all = hbpool_g.tile([128, 2048], BF16, tag="hb")
                    for hh in range(8):
                        nc.gpsimd.partition_broadcast(
                            hb_all[:, 256 * hh : 256 * hh + 256].bitcast(U32),
                            hr_all[0:1, 256 * hh : 256 * hh + 256].bitcast(U32),
                        )

                def emit_et(p):
                    for h in range(4):
                        for sc in range(2):
                            et = etpool.tile([128, 256], BF16, tag="et")
                            nc.vector.tensor_scalar(
                                et[:],
                                hb_all[:, 512 * h + 256 * p :][:, :256],
                                ac_sb[:, 8 * p + 4 * sc + h :][:, :1],
                                ac_sb[:, 16 + 8 * p + 4 * sc + h :][:, :1],
                                mybir.AluOpType.mult,
                                mybir.AluOpType.max,
                            )
                            nc.tensor.matmul(
                                agg[32 * h : 32 * h + 32,
                                    256 * p : 256 * p + 256],
                                featpad_g[p].ap()[
                                    :, 128 * sc + 32 * h :][:, :32],
                                et[:],
                                start=(sc == 0),
                                stop=(sc == 1),
                                tile_position=(0, 32 * h),
                            )

                if have_g:
                    emit_et(0)
                if s4 is not None:
                    nc.vector.reciprocal_approx_fast(rinv[:], den[:])
                if have_g:
                    emit_et(1)

                if s4 is not None:
                    nc.scalar.copy(rinv_bf[:], rinv[:])
                    nc.vector.tensor_tensor(
                        num_sc[:], num_sb[:], rinv_bf[:], mybir.AluOpType.mult
                    )
                    y_view = yt_s[:].rearrange(
                        "q (b wc f) -> q b wc f", b=4, wc=2
                    )
                    for wc in range(2):
                        out2 = psO.tile([128, 512], F32, tag="out2")
                        nc.tensor.matmul(
                            out2[:],
                            pwt_sb[:, 128 * wc : 128 * wc + 128],
                            num_sc[:],
                            start=True,
                            stop=True,
                        )
                        nc.scalar.copy(
                            y_view[:, bm_s[0] : bm_s[0] + 2, wc],
                            out2[:].rearrange("q (p f) -> q p f", p=2),
                        )
                    b0y = 2 * gp
                    yo = 512 * bm_s[0]
                    nc.gpsimd.dma_start(
                        y.ap()[b0y : b0y + 2].rearrange(
                            "b (wc q) f -> q (b wc) f", wc=2
                        ),
                        yt_s[:, yo : yo + 1024].rearrange(
                            "q (bwc f) -> q bwc f", bwc=4
                        ),
                    )

                if have_g:
                    pend.append((agg, y_t, bm, g))

    nc.compile()
    return nc


def host_prep(fc_w, attn_l, attn_r, gat_bias, proj_w, proj_b):
    fc_w = np.asarray(fc_w, np.float32)
    attn_l = np.asarray(attn_l, np.float32)
    attn_r = np.asarray(attn_r, np.float32)
    gat_bias = np.asarray(gat_bias, np.float32)
    proj_w = np.asarray(proj_w, np.float32)
    proj_b = np.asarray(proj_b, np.float32)

    fcl = np.einsum("hdw,hd->wh", fc_w.reshape(H, D, W), attn_l)
    c36 = np.concatenate([fc_w.T, fcl], axis=1).astype(np.float32)
    fcr4 = np.einsum("hdw,hd->wh", fc_w.reshape(H, D, W), attn_r).astype(np.float32)

    sel2r = np.zeros((128, 128), np.float32)
    for h in range(H):
        sel2r[32 * h + 8, 32 * h : 32 * h + 32] = 1.0

    pb2 = gat_bias @ proj_w.T + proj_b
    pwt = np.zeros((128, W), np.float32)
    for h in range(H):
        for j in range(D):
            pwt[32 * h + j, :] = proj_w[:, 8 * h + j]
    pwt[8, :] = pb2
    return c36, fcr4, sel2r, pwt


_CACHE = {}


def run(inputs, trace=False, trace_kwargs=None):
    x = np.asarray(inputs["x"], np.float32)
    c36, fcr4, sel2r, pwt = host_prep(
        inputs["fc_w"], inputs["attn_l"], inputs["attn_r"],
        inputs["gat_bias"], inputs["proj_w"], inputs["proj_b"],
    )
    if "nc" not in _CACHE:
        _CACHE["nc"] = build_nc(NB)
    nc = _CACHE["nc"]

    in_maps = []
    for c in range(NCORES):
        shard = np.ascontiguousarray(x[c * NB : (c + 1) * NB])
        in_maps.append(
            {"x": shard, "c36": c36, "fcr4": fcr4, "sel2r": sel2r, "pwt": pwt}
        )
    res = run_bass_kernel_spmd(
        nc, in_maps, core_ids=list(range(NCORES)), trace=trace,
        trace_kwargs=trace_kwargs or {},
    )
    y = np.concatenate([r["y"] for r in res.results], axis=0)
    return np.ascontiguousarray(y), res


def kernel(**inputs) -> np.ndarray:
    y, _ = run(inputs, trace=False)
    return y


# revision 5
# speedup vs baseline: 1.0211x; 1.0211x over previous
"""Trainium2 Bass kernel for nn_DGLFeatureGAT (dense per-batch GAT over
F=256 feature-nodes, window W=256, H=4 heads, D=8; data-parallel over
batch: 32 batches/core on 8 cores).

Math: alpha = softmax_s(leaky_relu(el_s + er_i)); softmax cancels any
dst-only factor, so Etil[s,i] = max(a_s*h_i, c_s) with a=exp(el),
c=exp(0.2*el), h=exp(0.8*er) — one DVE tensor_scalar per [128,256] tile.

Key performance structure (137 us on HW, vs 167 us baseline):
- x staged per-kc as float32r (no bf16 convert; fp32r matmuls run at
  bf16 speed when the moving side is >=256 wide)
- software pipeline: stage-4 (normalize/project/store) of group g-1 is
  emitted woven into stages 1-3 of group g (recip sits between the two
  8-op et half-blocks so the DVE never head-of-line blocks)
- h-broadcast: one merged DMA ([4,512]->[1,2048], partition-first APs)
  + 8 small int32-bitcast partition_broadcasts (short GpSimd SBUF-port
  locks; DVE PTR ops contend with GpSimd for the shared SBUF port pair)
- y written from per-group [128,1024] slices via the gpsimd DMA queue
- per-head hb slices ready early -> et starts sooner
"""
import sys
import numpy as np

sys.path.insert(0, "/opt/trn_rl_repo")

import concourse.bass as bass  # noqa: E402
import concourse.bacc as bacc  # noqa: E402
import concourse.tile as tile  # noqa: E402
from concourse import mybir  # noqa: E402
from concourse.bass_utils import run_bass_kernel_spmd  # noqa: E402

F32 = mybir.dt.float32
F32R = mybir.dt.float32r
BF16 = mybir.dt.bfloat16
U32 = mybir.dt.uint32

B, W, F, H, D = 256, 256, 256, 4, 8
NCORES = 8
NB = B // NCORES
NEG_SLOPE = 0.2


def build_nc(nb: int = NB):
    nc = bacc.Bacc("TRN2", target_bir_lowering=False, debug=False)

    x = nc.dram_tensor("x", [nb, W, F], F32, kind="ExternalInput")
    c36 = nc.dram_tensor("c36", [W, 36], F32, kind="ExternalInput")
    fcr4 = nc.dram_tensor("fcr4", [W, 4], F32, kind="ExternalInput")
    sel2r = nc.dram_tensor("sel2r", [128, 128], F32, kind="ExternalInput")
    pwt = nc.dram_tensor("pwt", [128, W], F32, kind="ExternalInput")
    y = nc.dram_tensor("y", [nb, W, F], F32, kind="ExternalOutput")

    featpads = [
        nc.alloc_sbuf_tensor(f"featpad{i}", [128, 256], BF16) for i in range(2)
    ]

    with tile.TileContext(nc) as tc:
        with (
            tc.tile_pool(name="consts", bufs=1) as cpool,
            tc.tile_pool(name="xp", bufs=2) as xpool,
            tc.tile_pool(name="yp", bufs=2) as ypool,
            tc.tile_pool(name="hbA", bufs=5) as hbpoolA,
            tc.tile_pool(name="hbB", bufs=5) as hbpoolB,
            tc.tile_pool(name="et", bufs=16) as etpool,
            tc.tile_pool(name="h4", bufs=4) as h4pool,
            tc.tile_pool(name="hr", bufs=8) as hrpool,
            tc.tile_pool(name="ac", bufs=4) as acpool,
            tc.tile_pool(name="nm", bufs=2) as nmpool,
            tc.tile_pool(name="psA", bufs=2, space="PSUM") as psA,
            tc.tile_pool(name="psE", bufs=1, space="PSUM") as psE,
            tc.tile_pool(name="psG", bufs=2, space="PSUM") as psG,
            tc.tile_pool(name="psD", bufs=1, space="PSUM") as psD,
            tc.tile_pool(name="psO", bufs=2, space="PSUM") as psO,
        ):
            # ---- constants
            c36_f32 = cpool.tile([128, 72], F32R)
            nc.sync.dma_start(
                c36_f32[:].rearrange("p (c n) -> p c n", c=2),
                c36.ap().rearrange("(c p) n -> p c n", c=2).bitcast(F32R),
            )
            fcr_f32 = cpool.tile([128, 8], F32R)
            nc.sync.dma_start(
                fcr_f32[:].rearrange("p (c n) -> p c n", c=2),
                fcr4.ap().rearrange("(c p) n -> p c n", c=2).bitcast(F32R),
            )
            sel2r_f32 = cpool.tile([128, 128], F32)
            nc.sync.dma_start(sel2r_f32[:], sel2r.ap())
            pwt_f32 = cpool.tile([128, 256], F32)
            nc.sync.dma_start(pwt_f32[:], pwt.ap())

            sel2r_sb = cpool.tile([128, 128], BF16)
            pwt_sb = cpool.tile([128, 256], BF16)

            for fp in featpads:
                nc.gpsimd.memset(fp.ap(), 0.0)
                ones_view = fp.ap().rearrange(
                    "q (sc h j) -> q sc h j", sc=2, h=4
                )[:, :, :, 8:9]
                nc.gpsimd.memset(ones_view, 1.0)

            x_bf = None
            y_t = None
            pend = []  # (agg, ac_unused, y_t, bm, g)
            ngroups = nb // 2
            for it in range(ngroups + 1):
                g = it
                have_g = g < ngroups
                s4 = pend.pop(0) if it >= 1 else None
                if it == 1:
                    nc.scalar.copy(sel2r_sb[:], sel2r_f32[:])
                    nc.scalar.copy(pwt_sb[:], pwt_f32[:])

                if have_g:
                    bm = [(2 * g + p) % 4 for p in range(2)]
                    if g % 2 == 0:
                        b0 = 2 * g
                        x_t0 = xpool.tile([128, 1024], F32R, tag="xt0")
                        x_t1 = xpool.tile([128, 1024], F32R, tag="xt1")
                        x_bf = [x_t0, x_t1]
                        for bp in range(2):
                            for kc in range(2):
                                nc.sync.dma_start(
                                    x_bf[kc][
                                        :, 512 * bp : 512 * bp + 512
                                    ].rearrange("q (b f) -> q b f", b=2),
                                    x.ap()[b0 + 2 * bp : b0 + 2 * bp + 2].rearrange(
                                        "b (kc q) f -> kc q b f", kc=2
                                    )[kc].bitcast(F32R),
                                )
                        y_t = ypool.tile([128, 2048], F32, tag="yt")

                    formA = psA.tile([128, 144], F32, tag="formA")
                    er_ps = psE.tile([128, 512], F32, tag="er")
                    agg = psG.tile([128, 512], F32, tag="agg")
                    ac_sb = acpool.tile([128, 32], F32, tag="ac")

                    # --- tensor: er first (longer downstream chain), then formA
                    for kc in range(2):
                        xo = 256 * bm[0]
                        nc.tensor.matmul(
                            er_ps[0:4, :],
                            fcr_f32[:, 4 * kc : 4 * kc + 4],
                            x_bf[kc][:, xo : xo + 512],
                            start=(kc == 0),
                            stop=(kc == 1),
                        )
                    for p in range(2):
                        for fh in range(2):
                            for kc in range(2):
                                xo = 256 * bm[p] + 128 * fh
                                nc.tensor.matmul(
                                    formA[:, 72 * p + 36 * fh :][:, :36],
                                    x_bf[kc][:, xo : xo + 128],
                                    c36_f32[:, 36 * kc : 36 * kc + 36],
                                    start=(kc == 0),
                                    stop=(kc == 1),
                                )

                    # --- scalar: exps + h4d + scatter; sync: hr
                    el_view = formA[:].rearrange(
                        "q (p fh n) -> q p fh n", p=2, fh=2
                    )[:, :, :, 32:36]
                    a_view = ac_sb[:, 0:16].rearrange(
                        "q (p fh n) -> q p fh n", p=2, fh=2
                    )
                    c_view = ac_sb[:, 16:32].rearrange(
                        "q (p fh n) -> q p fh n", p=2, fh=2
                    )
                    nc.scalar.activation(
                        a_view, el_view, mybir.ActivationFunctionType.Exp,
                        scale=1.0,
                    )
                    nc.scalar.activation(
                        c_view, el_view, mybir.ActivationFunctionType.Exp,
                        scale=NEG_SLOPE,
                    )
                    h4d = h4pool.tile([4, 512], BF16, tag="h4d")
                    nc.scalar.activation(
                        h4d[:],
                        er_ps[0:4, :],
                        mybir.ActivationFunctionType.Exp,
                        scale=1.0 - NEG_SLOPE,
                    )
                    hr_all = hrpool.tile([1, 2048], BF16, tag="hr")
                    nc.sync.dma_start(
                        hr_all[0:1, :].rearrange("o (h n) -> o h n", h=4),
                        h4d[:].rearrange("h (o n) -> h o n", o=1),
                    )
                    featpad_g = [featpads[p] for p in range(2)]
                    for p in range(2):
                        fa = formA[:, 72 * p : 72 * p + 72].rearrange(
                            "q (fh n) -> q fh n", fh=2
                        )[:, :, 0:32].rearrange("q fh (h d) -> q fh h d", h=4)
                        fp_view = featpad_g[p].ap().rearrange(
                            "q (sc h j) -> q sc h j", sc=2, h=4
                        )[:, :, :, 0:8]
                        nc.scalar.copy(fp_view, fa)

                # --- stage 4 (g-1) early part: num copy + den matmul
                if s4 is not None:
                    agg_s, yt_s, bm_s, gp = s4
                    den = psD.tile([128, 512], F32, tag="den")
                    num_sb = nmpool.tile([128, 512], BF16, tag="num")
                    rinv = nmpool.tile([128, 512], F32, tag="rinv")
                    rinv_bf = nmpool.tile([128, 512], BF16, tag="rinvb")
                    num_sc = nmpool.tile([128, 512], BF16, tag="nsc")
                    nc.scalar.copy(num_sb[:], agg_s[:])
                    nc.tensor.matmul(
                        den[:], sel2r_sb[:], num_sb[:], start=True, stop=True
                    )

                if have_g:
                    hbpool_g = hbpoolA if g % 2 == 0 else hbpoolB
                    hb_all = hbpool_g.tile([128, 2048], BF16, tag="hb")
                    for hh in range(8):
                        nc.gpsimd.partition_broadcast(
                            hb_all[:, 256 * hh : 256 * hh + 256].bitcast(U32),
                            hr_all[0:1, 256 * hh : 256 * hh + 256].bitcast(U32),
                        )

                def emit_et(p):
                    for h in range(4):
                        for sc in range(2):
                            et = etpool.tile([128, 256], BF16, tag="et")
                            nc.vector.tensor_scalar(
                                et[:],
                                hb_all[:, 512 * h + 256 * p :][:, :256],
                                ac_sb[:, 8 * p + 4 * sc + h :][:, :1],
                                ac_sb[:, 16 + 8 * p + 4 * sc + h :][:, :1],
                                mybir.AluOpType.mult,
                                mybir.AluOpType.max,
                            )
                            nc.tensor.matmul(
                                agg[32 * h : 32 * h + 32,
                                    256 * p : 256 * p + 256],
                                featpad_g[p].ap()[
                                    :, 128 * sc + 32 * h :][:, :32],
                                et[:],
                                start=(sc == 0),
                                stop=(sc == 1),
                                tile_position=(0, 32 * h),
                            )

                if have_g:
                    emit_et(0)
                if s4 is not None:
                    nc.vector.reciprocal_approx_fast(rinv[:], den[:])
                if have_g:
                    emit_et(1)

                if s4 is not None:
                    nc.scalar.copy(rinv_bf[:], rinv[:])
                    nc.vector.tensor_tensor(
                        num_sc[:], num_sb[:], rinv_bf[:], mybir.AluOpType.mult
                    )
                    y_view = yt_s[:].rearrange(
                        "q (b wc f) -> q b wc f", b=4, wc=2
                    )
                    for wc in range(2):
                        out2 = psO.tile([128, 512], F32, tag="out2")
                        nc.tensor.matmul(
                            out2[:],
                            pwt_sb[:, 128 * wc : 128 * wc + 128],
                            num_sc[:],
                            start=True,
                            stop=True,
                        )
                        nc.scalar.copy(
                            y_view[:, bm_s[0] : bm_s[0] + 2, wc],
                            out2[:].rearrange("q (p f) -> q p f", p=2),
                        )
                    b0y = 2 * gp
                    yo = 512 * bm_s[0]
                    nc.gpsimd.dma_start(
                        y.ap()[b0y : b0y + 2].rearrange(
                            "b (wc q) f -> q (b wc) f", wc=2
                        ),
                        yt_s[:, yo : yo + 1024].rearrange(
                            "q (bwc f) -> q bwc f", bwc=4
                        ),
                    )

                if have_g:
                    pend.append((agg, y_t, bm, g))

    nc.compile()
    return nc


def host_prep(fc_w, attn_l, attn_r, gat_bias, proj_w, proj_b):
    fc_w = np.asarray(fc_w, np.float32)
    attn_l = np.asarray(attn_l, np.float32)
    attn_r = np.asarray(attn_r, np.float32)
    gat_bias = np.asarray(gat_bias, np.float32)
    proj_w = np.asarray(proj_w, np.float32)
    proj_b = np.asarray(proj_b, np.float32)

    fcl = np.einsum("hdw,hd->wh", fc_w.reshape(H, D, W), attn_l)
    c36 = np.concatenate([fc_w.T, fcl], axis=1).astype(np.float32)
    fcr4 = np.einsum("hdw,hd->wh", fc_w.reshape(H, D, W), attn_r).astype(np.float32)

    sel2r = np.zeros((128, 128), np.float32)
    for h in range(H):
        sel2r[32 * h + 8, 32 * h : 32 * h + 32] = 1.0

    pb2 = gat_bias @ proj_w.T + proj_b
    pwt = np.zeros((128, W), np.float32)
    for h in range(H):
        for j in range(D):
            pwt[32 * h + j, :] = proj_w[:, 8 * h + j]
    pwt[8, :] = pb2
    return c36, fcr4, sel2r, pwt


_CACHE = {}


def run(inputs, trace=False, trace_kwargs=None):
    x = np.asarray(inputs["x"], np.float32)
    c36, fcr4, sel2r, pwt = host_prep(
        inputs["fc_w"], inputs["attn_l"], inputs["attn_r"],
        inputs["gat_bias"], inputs["proj_w"], inputs["proj_b"],
    )
    if "nc" not in _CACHE:
        _CACHE["nc"] = build_nc(NB)
    nc = _CACHE["nc"]

    in_maps = []
    for c in range(NCORES):
        shard = np.ascontiguousarray(x[c * NB : (c + 1) * NB])
        in_maps.append(
            {"x": shard, "c36": c36, "fcr4": fcr4, "sel2r": sel2r, "pwt": pwt}
        )
    res = run_bass_kernel_spmd(
        nc, in_maps, core_ids=list(range(NCORES)), trace=trace,
        trace_kwargs=trace_kwargs or {},
    )
    y = np.concatenate([r["y"] for r in res.results], axis=0)
    return np.ascontiguousarray(y), res


def kernel(**inputs) -> np.ndarray:
    y, _ = run(inputs, trace=False)
    return y
